# revision 12
# baseline (speedup 1.0000x reference)
"""CARAFE content-aware upsampling on 8 Trainium2 NeuronCores (Bass/Tile).

Problem: features (4,128,64,64) f32, masks (4,25,128,128) f32
         -> out (4,128,128,128) f32
out[n,c,2h+a,2w+b] = sum_{i,j in 5x5} f[n,c,h+i-2,w+j-2] * m[n,5i+j,2h+a,2w+b]

Strategy (per core = one (n, h-half) shard):
  For each low-res row h, out[c, (a, wup)] (2 upsampled rows x 128 cols)
  = 5 PSUM-accumulated fp16 matmuls, one per kernel-row i:
     out += fT_row(h+i-2).T @ B_i
  fT_row: W-padded transposed feature rows [w''(68), c(128)],
  B_i: banded mask matrix [w''(68), 256] with masks on diagonals.

  v5 design notes (from HW probe experiments):
  * The PE streams a CONTIGUOUS rhs ([.., [1, N]]) at ~109 ns per 256-col
    fp16 matmul (2.4 GHz warm), but any multi-dim strided rhs AP runs at
    ~228 ns.  So each job's five 256-col band windows are first
    compacted into a contiguous per-job tile jb[68, 5*256] by a single
    DVE/Pool copy (the copy engines handle strided APs at full rate),
    and every matmul rhs is then a plain slice of jb.
  * The scatter-facing band keeps the h-minor H-blocked layout (HB=8):
    diagonal runs of 20*HB fp16 = 320 B -> only 340 descriptors per
    8-job block, direct HBM->SBUF on the HWDGE queues.
  * Zero backgrounds (band + K-pad) are memset once per double buffer
    through a uint64-bitcast view (4x fewer memset elements).
  * Scatters for later blocks are emitted inline in the job loop so
    Tile's program-order tracking creates the WAR edges (scatter waits
    for the prior block's reformat reads).
"""
import sys

if "/opt/trn_rl_repo" not in sys.path:
    sys.path.insert(0, "/opt/trn_rl_repo")

from contextlib import ExitStack

import numpy as np

import concourse.tile as tile
from concourse import bacc, mybir
from concourse.ap import AP
from concourse.bass_utils import run_bass_kernel_spmd

# ---- problem constants (hardcoded per harness contract) ----
N = 4
C = 128
H = 64
W = 64
KS = 5
PAD = 2
SCALE = 2
WP = W + KS - 1          # 68 contraction rows
NB = SCALE * W           # 128 upsampled cols per hup row
RUN = 4 * KS             # 20 elems per diagonal run (w,b,a interleaved)
REG = 2 * NB + 32        # 288 per-band region: 16 pad | 256 data | 16 pad
NH = H // 2              # 32 low-res rows per core
NROWS = NH + 4           # 36 feature rows per shard (halo zero-padded)
HB = 8                   # jobs per band block (h-minor in band layout)
NBLK = NH // HB          # 4 blocks
BW2 = KS * REG * HB      # 11520 band buffer free width (fp16 elems)
BLKW = KS * RUN * HB     # 800 mask-run elems per block per partition
JBW = KS * 2 * NB        # 1280 contiguous per-job band width
OBATCH = 8               # jobs per output DMA

F16 = mybir.dt.float16
F32 = mybir.dt.float32
U32 = mybir.dt.uint32

_PROG_CACHE: dict = {}


def _device_body(tc, ctx, out_ap, ft_ap, msk_ap):
    nc = tc.nc
    sb = ctx.enter_context(tc.tile_pool(name="sb", bufs=1))
    psum = ctx.enter_context(tc.tile_pool(name="ps", bufs=6, space="PSUM"))
    obp = ctx.enter_context(tc.tile_pool(name="ob", bufs=2))
    jbp = ctx.enter_context(tc.tile_pool(name="jb", bufs=4))

    # features: first rows quickly on sync, rest on scalar
    ft = sb.tile([WP, NROWS * C], F16)
    nc.sync.dma_start(ft[:, : 9 * C], ft_ap[:, : 9 * C])
    nc.scalar.dma_start(ft[:, 9 * C :], ft_ap[:, 9 * C :])

    # h-minor band double buffer; zero background memset once through a
    # uint32 view (2x fewer elements), split across vector+gpsimd
    bb = [
        sb.tile([WP, BW2], F16, name=f"bb{q}", tag=f"bb{q}") for q in range(2)
    ]
    HKW = BW2 // 2 // 2    # half of the uint32-view width
    for q in range(2):
        v = bb[q][:].bitcast(U32)
        nc.vector.memset(v[:, :HKW], 0)
        nc.gpsimd.memset(v[:, HKW:], 0)

    def scatter(b):
        bap = bb[b % 2][:]
        dst = AP(
            bap.tensor,
            bap.offset,
            [[BW2 + 4 * HB, WP], [REG * HB, KS], [1, RUN * HB]],
        )
        src = AP(
            msk_ap.tensor,
            msk_ap.offset + b * BLKW,
            [[NBLK * BLKW, WP], [RUN * HB, KS], [1, RUN * HB]],
        )
        eng = (nc.sync, nc.scalar)[b % 2]
        eng.dma_start(dst, src)

    scatter(0)
    scatter(1)

    ob = None
    for hl in range(NH):
        b, p = divmod(hl, HB)
        if p == 0 and 2 <= b + 1 < NBLK:
            scatter(b + 1)
        bap = bb[b % 2][:]

        # compact this job's five 256-col windows into a contiguous tile
        jb = jbp.tile([WP, JBW], F16)
        src = AP(
            bap.tensor,
            bap.offset + 16 * HB + p,
            [[BW2, WP], [REG * HB, KS], [HB, 2], [2 * HB, NB]],
        )
        if hl % 2 == 0:
            nc.vector.tensor_copy(jb[:], src)
        else:
            nc.gpsimd.tensor_copy(jb[:], src)

        ps = psum.tile([C, 2 * NB], F32)
        for i in range(KS):
            lhsT = ft[:, (hl + i) * C : (hl + i + 1) * C]
            rhs = jb[:, i * 2 * NB : (i + 1) * 2 * NB]
            nc.tensor.matmul(ps[:], lhsT, rhs, start=(i == 0), stop=(i == 4))

        if hl % OBATCH == 0:
            ob = obp.tile([C, OBATCH * 2 * NB], F32)
        sl = ob[:, (hl % OBATCH) * 2 * NB : (hl % OBATCH + 1) * 2 * NB]
        if hl % 2 == 0:
            nc.scalar.copy(sl, ps[:])
        else:
            nc.vector.tensor_copy(sl, ps[:])

        if hl == NH - 5:
            g = hl - (OBATCH - 5)
            nc.gpsimd.dma_start(
                out_ap[:, 2 * g : 2 * g + 8, :], ob[:, : 4 * 2 * NB]
            )
        elif hl == NH - 1:
            nc.scalar.dma_start(
                out_ap[:, 2 * (NH - 4) : 2 * NH, :], ob[:, 4 * 2 * NB :]
            )
        elif hl % OBATCH == OBATCH - 1:
            g = hl - (OBATCH - 1)
            nc.gpsimd.dma_start(
                out_ap[:, 2 * g : 2 * g + 2 * OBATCH, :], ob[:]
            )


def _build_program():
    nc = bacc.Bacc(
        "TRN2", debug=False, enable_asserts=False, target_bir_lowering=False
    )
    ft_t = nc.dram_tensor("ft", [WP, NROWS * C], F16, kind="ExternalInput")
    msk_t = nc.dram_tensor("mskr", [WP, NBLK * BLKW], F16, kind="ExternalInput")
    out_t = nc.dram_tensor("out", [C, 2 * NH, NB], F32, kind="ExternalOutput")

    with tile.TileContext(nc) as tc, ExitStack() as ctx:
        _device_body(tc, ctx, out_t.ap(), ft_t.ap(), msk_t.ap())
    nc.compile()
    return nc


def _prep_ft(feat_n: np.ndarray, h0: int) -> np.ndarray:
    """[C,H,W] -> fT[w'', r, c] fp16 with r over [h0-2, h0+NH+2)."""
    ft = np.zeros((WP, NROWS, C), np.float16)
    r_lo, r_hi = h0 - 2, h0 + NH + 2
    s_lo, s_hi = max(r_lo, 0), min(r_hi, H)
    ft[PAD : PAD + W, s_lo - r_lo : s_hi - r_lo, :] = (
        feat_n[:, s_lo:s_hi, :].transpose(2, 1, 0).astype(np.float16)
    )
    return np.ascontiguousarray(ft.reshape(WP, NROWS * C))


def _prep_msk_full(masks_n: np.ndarray) -> np.ndarray:
    """[25, 2H, 2W] -> full[w', h, i, t20]  [WP, H, KS, RUN]
    t20 = (w - (w'-4))*4 + b*2 + a; value = masks[5i + (4 - t20//4), 2h+a, 2w+b]
    """
    tt = np.arange(RUN)
    wpp = np.arange(WP)
    dw = tt // 4
    b = (tt % 4) // 2
    a = tt % 2
    j = 4 - dw
    wup = 2 * (wpp[:, None] - 4 + dw[None, :]) + b[None, :]
    wup_c = np.clip(wup, 0, 2 * W - 1)                     # [WP, RUN]
    i_ar = np.arange(KS)
    k_full = 5 * i_ar[:, None] + j[None, :]                # [KS, RUN]
    hh = np.arange(H)
    hup = 2 * hh[:, None] + a[None, :]                     # [H, RUN]
    out = masks_n[
        k_full[None, None, :, :],
        hup[None, :, None, :],
        wup_c[:, None, None, :],
    ]  # [WP, H, KS, RUN]
    return out


def _prep_msk_runs(full: np.ndarray, h0: int) -> np.ndarray:
    """band-run stream [WP, NBLK*BLKW] fp16, order (w'', B, i, r, p)."""
    sl = full[:, h0 : h0 + NH]                              # [WP, NH, KS, RUN]
    arr = sl.reshape(WP, NBLK, HB, KS, RUN).transpose(0, 1, 3, 4, 2)
    return np.ascontiguousarray(
        arr.reshape(WP, NBLK * BLKW).astype(np.float16)
    )


def kernel(features: np.ndarray, masks: np.ndarray, _perf: dict | None = None):
    features = np.asarray(features, dtype=np.float32)
    masks = np.asarray(masks, dtype=np.float32)

    if "nc" not in _PROG_CACHE:
        _PROG_CACHE["nc"] = _build_program()
    nc = _PROG_CACHE["nc"]

    in_maps = []
    fulls = [_prep_msk_full(masks[n]) for n in range(N)]
    for core in range(8):
        n, half = divmod(core, 2)
        h0 = NH * half
        in_maps.append(
            {
                "ft": _prep_ft(features[n], h0),
                "mskr": _prep_msk_runs(fulls[n], h0),
            }
        )

    trace = bool(_perf is not None and _perf.get("trace"))
    res = run_bass_kernel_spmd(
        nc, in_maps, core_ids=list(range(8)), trace=trace,
        **({} if not trace else {"trace_cores": [0]}),
    )
    if _perf is not None:
        _perf["exec_time_ns"] = res.exec_time_ns
        _perf["trace"] = res.instructions_and_trace

    out = np.empty((N, C, SCALE * H, SCALE * W), np.float32)
    for core in range(8):
        n, half = divmod(core, 2)
        out[n, :, 64 * half : 64 * half + 64, :] = res.results[core]["out"]
    return out


# revision 13
# speedup vs baseline: 1.5017x; 1.5017x over previous
"""CARAFE content-aware upsampling on 8 Trainium2 NeuronCores (Bass/Tile).

Problem: features (4,128,64,64) f32, masks (4,25,128,128) f32
         -> out (4,128,128,128) f32
out[n,c,2h+a,2w+b] = sum_{i,j in 5x5} f[n,c,h+i-2,w+j-2] * m[n,5i+j,2h+a,2w+b]

Strategy (per core = one (n, h-half) shard):
  For each low-res row h, out[c, (a, wup)] (2 upsampled rows x 128 cols)
  = 5 PSUM-accumulated fp16 matmuls, one per kernel-row i:
     out += fT_row(h+i-2).T @ B_i
  fT_row: W-padded transposed feature rows [w''(68), c(128)],
  B_i: banded mask matrix [w''(68), 256] with masks on diagonals.

  v5 design notes (from HW probe experiments):
  * The PE streams a CONTIGUOUS rhs ([.., [1, N]]) at ~109 ns per 256-col
    fp16 matmul (2.4 GHz warm), but any multi-dim strided rhs AP runs at
    ~228 ns.  So each job's five 256-col band windows are first
    compacted into a contiguous per-job tile jb[68, 5*256] by a single
    DVE/Pool copy (the copy engines handle strided APs at full rate),
    and every matmul rhs is then a plain slice of jb.
  * The scatter-facing band keeps the h-minor H-blocked layout (HB=8):
    diagonal runs of 20*HB fp16 = 320 B -> only 340 descriptors per
    8-job block, direct HBM->SBUF on the HWDGE queues.
  * Zero backgrounds (band + K-pad) are memset once per double buffer
    through a uint64-bitcast view (4x fewer memset elements).
  * Scatters for later blocks are emitted inline in the job loop so
    Tile's program-order tracking creates the WAR edges (scatter waits
    for the prior block's reformat reads).
"""
import sys

if "/opt/trn_rl_repo" not in sys.path:
    sys.path.insert(0, "/opt/trn_rl_repo")

from contextlib import ExitStack

import numpy as np

import concourse.tile as tile
from concourse import bacc, mybir
from concourse.ap import AP
from concourse.bass_utils import run_bass_kernel_spmd

# ---- problem constants (hardcoded per harness contract) ----
N = 4
C = 128
H = 64
W = 64
KS = 5
PAD = 2
SCALE = 2
WP = W + KS - 1          # 68 contraction rows
NB = SCALE * W           # 128 upsampled cols per hup row
RUN = 4 * KS             # 20 elems per diagonal run (w,b,a interleaved)
REG = 2 * NB + 32        # 288 per-band region: 16 pad | 256 data | 16 pad
NH = H // 2              # 32 low-res rows per core
NROWS = NH + 4           # 36 feature rows per shard (halo zero-padded)
HB = 2                   # jobs per band block (h-minor in band layout)
NBLK = NH // HB          # 16 blocks
BW2 = KS * REG * HB      # 11520 band buffer free width (fp16 elems)
BLKW = KS * RUN * HB     # 800 mask-run elems per block per partition
JBW = KS * 2 * NB        # 1280 contiguous per-job band width
OBATCH = 8               # jobs per output DMA

F16 = mybir.dt.float16
F32 = mybir.dt.float32
U32 = mybir.dt.uint32

_PROG_CACHE: dict = {}


def _device_body(tc, ctx, out_ap, ft_ap, msk_ap):
    nc = tc.nc
    sb = ctx.enter_context(tc.tile_pool(name="sb", bufs=1))
    psum = ctx.enter_context(tc.tile_pool(name="ps", bufs=6, space="PSUM"))
    obp = ctx.enter_context(tc.tile_pool(name="ob", bufs=2))

    # features: first rows quickly on sync, rest on scalar
    ft = sb.tile([WP, NROWS * C], F16)
    nc.sync.dma_start(ft[:, : 9 * C], ft_ap[:, : 9 * C])
    nc.scalar.dma_start(ft[:, 9 * C :], ft_ap[:, 9 * C :])

    # h-minor band double buffer; zero background memset once through a
    # uint32 view (2x fewer elements), split across vector+gpsimd
    bb = [
        sb.tile([WP, BW2], F16, name=f"bb{q}", tag=f"bb{q}") for q in range(2)
    ]
    HKW = BW2 // 2 // 2    # half of the uint32-view width
    for q in range(2):
        v = bb[q][:].bitcast(U32)
        nc.vector.memset(v[:, :HKW], 0)
        nc.gpsimd.memset(v[:, HKW:], 0)

    def scatter(b):
        bap = bb[b % 2][:]
        dst = AP(
            bap.tensor,
            bap.offset,
            [[BW2 + 4 * HB, WP], [REG * HB, KS], [1, RUN * HB]],
        )
        src = AP(
            msk_ap.tensor,
            msk_ap.offset + b * BLKW,
            [[NBLK * BLKW, WP], [RUN * HB, KS], [1, RUN * HB]],
        )
        eng = (nc.sync, nc.scalar)[b % 2]
        eng.dma_start(dst, src)

    scatter(0)
    scatter(1)

    ob = None
    for hl in range(NH):
        b, p = divmod(hl, HB)
        if p == 0 and 2 <= b + 1 < NBLK:
            scatter(b + 1)
        bap = bb[b % 2][:]

        # single-strided rhs (4 B column step = full PE stream rate);
        # psum columns come out in band order (w, b, a)
        ps = psum.tile([C, 2 * NB], F32)
        for i in range(KS):
            lhsT = ft[:, (hl + i) * C : (hl + i + 1) * C]
            rhs = AP(
                bap.tensor,
                bap.offset + (i * REG + 16) * HB + p,
                [[BW2, WP], [HB, 2 * NB]],
            )
            nc.tensor.matmul(ps[:], lhsT, rhs, start=(i == 0), stop=(i == 4))

        if hl % OBATCH == 0:
            ob = obp.tile([C, OBATCH * 2 * NB], F32)
        sl = ob[:, (hl % OBATCH) * 2 * NB : (hl % OBATCH + 1) * 2 * NB]
        # strided psum read reorders (w, b, a) -> (a, wup) for the output
        psr = AP(ps.tensor, ps.offset, [[2 * NB, C], [1, 2], [2, 2 * NB // 2]])
        if hl % 2 == 0:
            nc.scalar.copy(sl, psr)
        else:
            nc.vector.tensor_copy(sl, psr)

        if hl == NH - 5:
            g = hl - (OBATCH - 5)
            nc.gpsimd.dma_start(
                out_ap[:, 2 * g : 2 * g + 8, :], ob[:, : 4 * 2 * NB]
            )
        elif hl == NH - 1:
            nc.scalar.dma_start(
                out_ap[:, 2 * (NH - 4) : 2 * NH, :], ob[:, 4 * 2 * NB :]
            )
        elif hl % OBATCH == OBATCH - 1:
            g = hl - (OBATCH - 1)
            nc.gpsimd.dma_start(
                out_ap[:, 2 * g : 2 * g + 2 * OBATCH, :], ob[:]
            )


def _build_program():
    nc = bacc.Bacc(
        "TRN2", debug=False, enable_asserts=False, target_bir_lowering=False
    )
    ft_t = nc.dram_tensor("ft", [WP, NROWS * C], F16, kind="ExternalInput")
    msk_t = nc.dram_tensor("mskr", [WP, NBLK * BLKW], F16, kind="ExternalInput")
    out_t = nc.dram_tensor("out", [C, 2 * NH, NB], F32, kind="ExternalOutput")

    with tile.TileContext(nc) as tc, ExitStack() as ctx:
        _device_body(tc, ctx, out_t.ap(), ft_t.ap(), msk_t.ap())
    nc.compile()
    return nc


def _prep_ft(feat_n: np.ndarray, h0: int) -> np.ndarray:
    """[C,H,W] -> fT[w'', r, c] fp16 with r over [h0-2, h0+NH+2)."""
    ft = np.zeros((WP, NROWS, C), np.float16)
    r_lo, r_hi = h0 - 2, h0 + NH + 2
    s_lo, s_hi = max(r_lo, 0), min(r_hi, H)
    ft[PAD : PAD + W, s_lo - r_lo : s_hi - r_lo, :] = (
        feat_n[:, s_lo:s_hi, :].transpose(2, 1, 0).astype(np.float16)
    )
    return np.ascontiguousarray(ft.reshape(WP, NROWS * C))


def _prep_msk_full(masks_n: np.ndarray) -> np.ndarray:
    """[25, 2H, 2W] -> full[w', h, i, t20]  [WP, H, KS, RUN]
    t20 = (w - (w'-4))*4 + b*2 + a; value = masks[5i + (4 - t20//4), 2h+a, 2w+b]
    """
    tt = np.arange(RUN)
    wpp = np.arange(WP)
    dw = tt // 4
    b = (tt % 4) // 2
    a = tt % 2
    j = 4 - dw
    wup = 2 * (wpp[:, None] - 4 + dw[None, :]) + b[None, :]
    wup_c = np.clip(wup, 0, 2 * W - 1)                     # [WP, RUN]
    i_ar = np.arange(KS)
    k_full = 5 * i_ar[:, None] + j[None, :]                # [KS, RUN]
    hh = np.arange(H)
    hup = 2 * hh[:, None] + a[None, :]                     # [H, RUN]
    out = masks_n[
        k_full[None, None, :, :],
        hup[None, :, None, :],
        wup_c[:, None, None, :],
    ]  # [WP, H, KS, RUN]
    return out


def _prep_msk_runs(full: np.ndarray, h0: int) -> np.ndarray:
    """band-run stream [WP, NBLK*BLKW] fp16, order (w'', B, i, r, p)."""
    sl = full[:, h0 : h0 + NH]                              # [WP, NH, KS, RUN]
    arr = sl.reshape(WP, NBLK, HB, KS, RUN).transpose(0, 1, 3, 4, 2)
    return np.ascontiguousarray(
        arr.reshape(WP, NBLK * BLKW).astype(np.float16)
    )


def kernel(features: np.ndarray, masks: np.ndarray, _perf: dict | None = None):
    features = np.asarray(features, dtype=np.float32)
    masks = np.asarray(masks, dtype=np.float32)

    if "nc" not in _PROG_CACHE:
        _PROG_CACHE["nc"] = _build_program()
    nc = _PROG_CACHE["nc"]

    in_maps = []
    fulls = [_prep_msk_full(masks[n]) for n in range(N)]
    for core in range(8):
        n, half = divmod(core, 2)
        h0 = NH * half
        in_maps.append(
            {
                "ft": _prep_ft(features[n], h0),
                "mskr": _prep_msk_runs(fulls[n], h0),
            }
        )

    trace = bool(_perf is not None and _perf.get("trace"))
    res = run_bass_kernel_spmd(
        nc, in_maps, core_ids=list(range(8)), trace=trace,
        **({} if not trace else {"trace_cores": [0]}),
    )
    if _perf is not None:
        _perf["exec_time_ns"] = res.exec_time_ns
        _perf["trace"] = res.instructions_and_trace

    out = np.empty((N, C, SCALE * H, SCALE * W), np.float32)
    for core in range(8):
        n, half = divmod(core, 2)
        out[n, :, 64 * half : 64 * half + 64, :] = res.results[core]["out"]
    return out


# revision 15
# speedup vs baseline: 1.5648x; 1.0420x over previous
"""CARAFE content-aware upsampling on 8 Trainium2 NeuronCores (Bass/Tile).

Problem: features (4,128,64,64) f32, masks (4,25,128,128) f32
         -> out (4,128,128,128) f32
out[n,c,2h+a,2w+b] = sum_{i,j in 5x5} f[n,c,h+i-2,w+j-2] * m[n,5i+j,2h+a,2w+b]

Strategy (per core = one (n, h-half) shard):
  For each low-res row h, out[c, (a, wup)] (2 upsampled rows x 128 cols)
  = 5 PSUM-accumulated fp16 matmuls, one per kernel-row i:
     out += fT_row(h+i-2).T @ B_i
  fT_row: W-padded transposed feature rows [w''(68), c(128)],
  B_i: banded mask matrix [w''(68), 256] with masks on diagonals.

  v5 design notes (from HW probe experiments):
  * The PE streams a CONTIGUOUS rhs ([.., [1, N]]) at ~109 ns per 256-col
    fp16 matmul (2.4 GHz warm), but any multi-dim strided rhs AP runs at
    ~228 ns.  So each job's five 256-col band windows are first
    compacted into a contiguous per-job tile jb[68, 5*256] by a single
    DVE/Pool copy (the copy engines handle strided APs at full rate),
    and every matmul rhs is then a plain slice of jb.
  * The scatter-facing band keeps the h-minor H-blocked layout (HB=8):
    diagonal runs of 20*HB fp16 = 320 B -> only 340 descriptors per
    8-job block, direct HBM->SBUF on the HWDGE queues.
  * Zero backgrounds (band + K-pad) are memset once per double buffer
    through a uint64-bitcast view (4x fewer memset elements).
  * Scatters for later blocks are emitted inline in the job loop so
    Tile's program-order tracking creates the WAR edges (scatter waits
    for the prior block's reformat reads).
"""
import sys

if "/opt/trn_rl_repo" not in sys.path:
    sys.path.insert(0, "/opt/trn_rl_repo")

from contextlib import ExitStack

import numpy as np

import concourse.tile as tile
from concourse import bacc, mybir
from concourse.ap import AP
from concourse.bass_utils import run_bass_kernel_spmd

# ---- problem constants (hardcoded per harness contract) ----
N = 4
C = 128
H = 64
W = 64
KS = 5
PAD = 2
SCALE = 2
WP = W + KS - 1          # 68 real contraction rows
KP = 128                 # zero-padded contraction (stride-2 rhs needs full K)
NB = SCALE * W           # 128 upsampled cols per hup row
RUN = 4 * KS             # 20 elems per diagonal run (w,b,a interleaved)
REG = 2 * NB + 32        # 288 per-band region: 16 pad | 256 data | 16 pad
NH = H // 2              # 32 low-res rows per core
NROWS = NH + 4           # 36 feature rows per shard (halo zero-padded)
HB = 2                   # jobs per band block (h-minor in band layout)
NBLK = NH // HB          # 16 blocks
BW2 = KS * REG * HB      # 11520 band buffer free width (fp16 elems)
BLKW = KS * RUN * HB     # 800 mask-run elems per block per partition
JBW = KS * 2 * NB        # 1280 contiguous per-job band width
OBATCH = 8               # jobs per output DMA

F16 = mybir.dt.float16
F32 = mybir.dt.float32
U32 = mybir.dt.uint32

_PROG_CACHE: dict = {}


def _device_body(tc, ctx, out_ap, ft_ap, msk_ap):
    nc = tc.nc
    sb = ctx.enter_context(tc.tile_pool(name="sb", bufs=1))
    psum = ctx.enter_context(tc.tile_pool(name="ps", bufs=6, space="PSUM"))
    obp = ctx.enter_context(tc.tile_pool(name="ob", bufs=2))

    # features with contraction rows 68-127 zero-padded.  Engines address
    # partitions at 32-aligned bases, so memset [64:128) FIRST, then the
    # row loads overwrite the real rows 64-67.
    ft = sb.tile([KP, NROWS * C], F16)
    fv = ft[64:KP, :].bitcast(U32)
    FKW = (NROWS * C) // 2 // 2
    nc.vector.memset(fv[:, :FKW], 0)
    nc.gpsimd.memset(fv[:, FKW:], 0)
    nc.sync.dma_start(ft[:WP, : 9 * C], ft_ap[:, : 9 * C])
    nc.scalar.dma_start(ft[:WP, 9 * C :], ft_ap[:, 9 * C :])

    # h-minor band double buffer (incl. K-pad rows); zero background
    # memset once through a uint32 view, split across vector+gpsimd
    bb = [
        sb.tile([KP, BW2], F16, name=f"bb{q}", tag=f"bb{q}") for q in range(2)
    ]
    HKW = BW2 // 2 // 2    # half of the uint32-view width
    for q in range(2):
        v = bb[q][:].bitcast(U32)
        nc.vector.memset(v[:, :HKW], 0)
        nc.gpsimd.memset(v[:, HKW:], 0)

    def scatter(b):
        bap = bb[b % 2][:]
        dst = AP(
            bap.tensor,
            bap.offset,
            [[BW2 + 4 * HB, WP], [REG * HB, KS], [1, RUN * HB]],
        )
        src = AP(
            msk_ap.tensor,
            msk_ap.offset + b * BLKW,
            [[NBLK * BLKW, WP], [RUN * HB, KS], [1, RUN * HB]],
        )
        eng = (nc.sync, nc.scalar)[b % 2]
        eng.dma_start(dst, src)

    scatter(0)
    scatter(1)

    ob = None
    for hl in range(NH):
        b, p = divmod(hl, HB)
        if p == 0 and 2 <= b + 1 < NBLK:
            scatter(b + 1)
        bap = bb[b % 2][:]

        # single-strided rhs (4 B column step = full PE stream rate);
        # psum columns come out in band order (w, b, a)
        ps = psum.tile([C, 2 * NB], F32)
        for i in range(KS):
            lhsT = ft[:, (hl + i) * C : (hl + i + 1) * C]
            rhs = AP(
                bap.tensor,
                bap.offset + (i * REG + 16) * HB + p,
                [[BW2, KP], [HB, 2 * NB]],
            )
            nc.tensor.matmul(ps[:], lhsT, rhs, start=(i == 0), stop=(i == 4))

        if hl % OBATCH == 0:
            ob = obp.tile([C, OBATCH * 2 * NB], F32)
        sl = ob[:, (hl % OBATCH) * 2 * NB : (hl % OBATCH + 1) * 2 * NB]
        # strided psum read reorders (w, b, a) -> (a, wup) for the output
        psr = AP(ps.tensor, ps.offset, [[2 * NB, C], [1, 2], [2, 2 * NB // 2]])
        if hl % 2 == 0:
            nc.scalar.copy(sl, psr)
        else:
            nc.vector.tensor_copy(sl, psr)

        if hl == NH - 5:
            g = hl - (OBATCH - 5)
            nc.gpsimd.dma_start(
                out_ap[:, 2 * g : 2 * g + 8, :], ob[:, : 4 * 2 * NB]
            )
        elif hl == NH - 1:
            nc.scalar.dma_start(
                out_ap[:, 2 * (NH - 4) : 2 * NH, :], ob[:, 4 * 2 * NB :]
            )
        elif hl % OBATCH == OBATCH - 1:
            g = hl - (OBATCH - 1)
            nc.gpsimd.dma_start(
                out_ap[:, 2 * g : 2 * g + 2 * OBATCH, :], ob[:]
            )


def _build_program():
    nc = bacc.Bacc(
        "TRN2", debug=False, enable_asserts=False, target_bir_lowering=False
    )
    ft_t = nc.dram_tensor("ft", [WP, NROWS * C], F16, kind="ExternalInput")
    msk_t = nc.dram_tensor("mskr", [WP, NBLK * BLKW], F16, kind="ExternalInput")
    out_t = nc.dram_tensor("out", [C, 2 * NH, NB], F32, kind="ExternalOutput")

    with tile.TileContext(nc) as tc, ExitStack() as ctx:
        _device_body(tc, ctx, out_t.ap(), ft_t.ap(), msk_t.ap())
    nc.compile()
    return nc


def _prep_ft(feat_n: np.ndarray, h0: int) -> np.ndarray:
    """[C,H,W] -> fT[w'', r, c] fp16 with r over [h0-2, h0+NH+2)."""
    ft = np.zeros((WP, NROWS, C), np.float16)
    r_lo, r_hi = h0 - 2, h0 + NH + 2
    s_lo, s_hi = max(r_lo, 0), min(r_hi, H)
    ft[PAD : PAD + W, s_lo - r_lo : s_hi - r_lo, :] = (
        feat_n[:, s_lo:s_hi, :].transpose(2, 1, 0).astype(np.float16)
    )
    return np.ascontiguousarray(ft.reshape(WP, NROWS * C))


def _prep_msk_full(masks_n: np.ndarray) -> np.ndarray:
    """[25, 2H, 2W] -> full[w', h, i, t20]  [WP, H, KS, RUN]
    t20 = (w - (w'-4))*4 + b*2 + a; value = masks[5i + (4 - t20//4), 2h+a, 2w+b]
    """
    tt = np.arange(RUN)
    wpp = np.arange(WP)
    dw = tt // 4
    b = (tt % 4) // 2
    a = tt % 2
    j = 4 - dw
    wup = 2 * (wpp[:, None] - 4 + dw[None, :]) + b[None, :]
    wup_c = np.clip(wup, 0, 2 * W - 1)                     # [WP, RUN]
    i_ar = np.arange(KS)
    k_full = 5 * i_ar[:, None] + j[None, :]                # [KS, RUN]
    hh = np.arange(H)
    hup = 2 * hh[:, None] + a[None, :]                     # [H, RUN]
    out = masks_n[
        k_full[None, None, :, :],
        hup[None, :, None, :],
        wup_c[:, None, None, :],
    ]  # [WP, H, KS, RUN]
    return out


def _prep_msk_runs(full: np.ndarray, h0: int) -> np.ndarray:
    """band-run stream [WP, NBLK*BLKW] fp16, order (w'', B, i, r, p)."""
    sl = full[:, h0 : h0 + NH]                              # [WP, NH, KS, RUN]
    arr = sl.reshape(WP, NBLK, HB, KS, RUN).transpose(0, 1, 3, 4, 2)
    return np.ascontiguousarray(
        arr.reshape(WP, NBLK * BLKW).astype(np.float16)
    )


def kernel(features: np.ndarray, masks: np.ndarray, _perf: dict | None = None):
    features = np.asarray(features, dtype=np.float32)
    masks = np.asarray(masks, dtype=np.float32)

    if "nc" not in _PROG_CACHE:
        _PROG_CACHE["nc"] = _build_program()
    nc = _PROG_CACHE["nc"]

    in_maps = []
    fulls = [_prep_msk_full(masks[n]) for n in range(N)]
    for core in range(8):
        n, half = divmod(core, 2)
        h0 = NH * half
        in_maps.append(
            {
                "ft": _prep_ft(features[n], h0),
                "mskr": _prep_msk_runs(fulls[n], h0),
            }
        )

    trace = bool(_perf is not None and _perf.get("trace"))
    res = run_bass_kernel_spmd(
        nc, in_maps, core_ids=list(range(8)), trace=trace,
        **({} if not trace else {"trace_cores": [0]}),
    )
    if _perf is not None:
        _perf["exec_time_ns"] = res.exec_time_ns
        _perf["trace"] = res.instructions_and_trace

    out = np.empty((N, C, SCALE * H, SCALE * W), np.float32)
    for core in range(8):
        n, half = divmod(core, 2)
        out[n, :, 64 * half : 64 * half + 64, :] = res.results[core]["out"]
    return out


# revision 16
# speedup vs baseline: 2.1133x; 1.3505x over previous
"""CARAFE content-aware upsampling on 8 Trainium2 NeuronCores (Bass/Tile).

Problem: features (4,128,64,64) f32, masks (4,25,128,128) f32
         -> out (4,128,128,128) f32
out[n,c,2h+a,2w+b] = sum_{i,j in 5x5} f[n,c,h+i-2,w+j-2] * m[n,5i+j,2h+a,2w+b]

Strategy (per core = one (n, h-half) shard):
  For each low-res row h, out[c, (a, wup)] (2 upsampled rows x 128 cols)
  = 5 PSUM-accumulated fp16 matmuls, one per kernel-row i:
     out += fT_row(h+i-2).T @ B_i
  fT_row: W-padded transposed feature rows [w''(68), c(128)],
  B_i: banded mask matrix [w''(68), 256] with masks on diagonals.

  v5 design notes (from HW probe experiments):
  * The PE streams a CONTIGUOUS rhs ([.., [1, N]]) at ~109 ns per 256-col
    fp16 matmul (2.4 GHz warm), but any multi-dim strided rhs AP runs at
    ~228 ns.  So each job's five 256-col band windows are first
    compacted into a contiguous per-job tile jb[68, 5*256] by a single
    DVE/Pool copy (the copy engines handle strided APs at full rate),
    and every matmul rhs is then a plain slice of jb.
  * The scatter-facing band keeps the h-minor H-blocked layout (HB=8):
    diagonal runs of 20*HB fp16 = 320 B -> only 340 descriptors per
    8-job block, direct HBM->SBUF on the HWDGE queues.
  * Zero backgrounds (band + K-pad) are memset once per double buffer
    through a uint64-bitcast view (4x fewer memset elements).
  * Scatters for later blocks are emitted inline in the job loop so
    Tile's program-order tracking creates the WAR edges (scatter waits
    for the prior block's reformat reads).
"""
import sys

if "/opt/trn_rl_repo" not in sys.path:
    sys.path.insert(0, "/opt/trn_rl_repo")

from contextlib import ExitStack

import numpy as np

import concourse.tile as tile
from concourse import bacc, mybir
from concourse.ap import AP
from concourse.bass_utils import run_bass_kernel_spmd

# ---- problem constants (hardcoded per harness contract) ----
N = 4
C = 128
H = 64
W = 64
KS = 5
PAD = 2
SCALE = 2
WP = W + KS - 1          # 68 real contraction rows
KP = 128                 # zero-padded contraction (stride-2 rhs needs full K)
NB = SCALE * W           # 128 upsampled cols per hup row
RUN = 4 * KS             # 20 elems per diagonal run (w,b,a interleaved)
REG = 2 * NB + 32        # 288 per-band region: 16 pad | 256 data | 16 pad
NH = H // 2              # 32 low-res rows per core
NROWS = NH + 4           # 36 feature rows per shard (halo zero-padded)
HB = 2                   # jobs per band block (h-minor in band layout)
NBLK = NH // HB          # 16 blocks
BW2 = KS * REG * HB      # 11520 band buffer free width (fp16 elems)
BLKW = KS * RUN * HB     # 800 mask-run elems per block per partition
JBW = KS * 2 * NB        # 1280 contiguous per-job band width
OBATCH = 8               # jobs per output DMA

F16 = mybir.dt.float16
F32 = mybir.dt.float32
U32 = mybir.dt.uint32

_PROG_CACHE: dict = {}


def _device_body(tc, ctx, out_ap, ft_ap, msk_ap):
    nc = tc.nc
    sb = ctx.enter_context(tc.tile_pool(name="sb", bufs=1))
    psum = ctx.enter_context(tc.tile_pool(name="ps", bufs=6, space="PSUM"))
    obp = ctx.enter_context(tc.tile_pool(name="ob", bufs=2))

    # features with contraction rows 68-127 zero-padded.  Engines address
    # partitions at 32-aligned bases, so memset [64:128) FIRST, then the
    # row loads overwrite the real rows 64-67.
    ft = sb.tile([KP, NROWS * C], F16)
    fv = ft[64:KP, :].bitcast(U32)
    FKW = (NROWS * C) // 2 // 2
    nc.vector.memset(fv[:, :FKW], 0)
    nc.gpsimd.memset(fv[:, FKW:], 0)
    nc.sync.dma_start(ft[:WP, : 9 * C], ft_ap[:, : 9 * C])

    # h-minor band buffers, 6-deep so the per-block diagonal scatters
    # (~2.3 us each on the two HWDGE queues) run 5+ blocks ahead of the
    # PE; zero background memset once via uint32 views.
    NBB = 6
    bb = [
        sb.tile([KP, BW2], F16, name=f"bb{q}", tag=f"bb{q}")
        for q in range(NBB)
    ]
    HKW = BW2 // 2 // 2    # half of the uint32-view width
    for q in range(NBB):
        v = bb[q][:].bitcast(U32)
        nc.vector.memset(v[:, :HKW], 0)
        nc.gpsimd.memset(v[:, HKW:], 0)

    def scatter(b):
        bap = bb[b % NBB][:]
        dst = AP(
            bap.tensor,
            bap.offset,
            [[BW2 + 4 * HB, WP], [REG * HB, KS], [1, RUN * HB]],
        )
        src = AP(
            msk_ap.tensor,
            msk_ap.offset + b * BLKW,
            [[NBLK * BLKW, WP], [RUN * HB, KS], [1, RUN * HB]],
        )
        eng = (nc.sync, nc.scalar)[b % 2]
        eng.dma_start(dst, src)

    # first six blocks' scatters upfront (fresh buffers, no WAR), with
    # the remaining feature rows interleaved on the scalar queue
    for b0 in range(3):
        scatter(2 * b0)
        scatter(2 * b0 + 1)
        if b0 == 0:
            nc.scalar.dma_start(
                ft[:WP, 9 * C : 22 * C], ft_ap[:, 9 * C : 22 * C]
            )
        elif b0 == 1:
            nc.scalar.dma_start(ft[:WP, 22 * C :], ft_ap[:, 22 * C :])

    ob = None
    for hl in range(NH):
        b, p = divmod(hl, HB)
        if p == 0 and NBB <= b + 5 < NBLK:
            scatter(b + 5)
        bap = bb[b % NBB][:]

        # single-strided rhs (4 B column step = full PE stream rate);
        # psum columns come out in band order (w, b, a)
        ps = psum.tile([C, 2 * NB], F32)
        for i in range(KS):
            lhsT = ft[:, (hl + i) * C : (hl + i + 1) * C]
            rhs = AP(
                bap.tensor,
                bap.offset + (i * REG + 16) * HB + p,
                [[BW2, KP], [HB, 2 * NB]],
            )
            nc.tensor.matmul(ps[:], lhsT, rhs, start=(i == 0), stop=(i == 4))

        if hl % OBATCH == 0:
            ob = obp.tile([C, OBATCH * 2 * NB], F32)
        sl = ob[:, (hl % OBATCH) * 2 * NB : (hl % OBATCH + 1) * 2 * NB]
        # strided psum read reorders (w, b, a) -> (a, wup) for the output
        psr = AP(ps.tensor, ps.offset, [[2 * NB, C], [1, 2], [2, 2 * NB // 2]])
        if hl % 2 == 0:
            nc.scalar.copy(sl, psr)
        else:
            nc.vector.tensor_copy(sl, psr)

        if hl == NH - 5:
            g = hl - (OBATCH - 5)
            nc.gpsimd.dma_start(
                out_ap[:, 2 * g : 2 * g + 8, :], ob[:, : 4 * 2 * NB]
            )
        elif hl == NH - 1:
            nc.scalar.dma_start(
                out_ap[:, 2 * (NH - 4) : 2 * NH, :], ob[:, 4 * 2 * NB :]
            )
        elif hl % OBATCH == OBATCH - 1:
            g = hl - (OBATCH - 1)
            nc.gpsimd.dma_start(
                out_ap[:, 2 * g : 2 * g + 2 * OBATCH, :], ob[:]
            )


def _build_program():
    nc = bacc.Bacc(
        "TRN2", debug=False, enable_asserts=False, target_bir_lowering=False
    )
    ft_t = nc.dram_tensor("ft", [WP, NROWS * C], F16, kind="ExternalInput")
    msk_t = nc.dram_tensor("mskr", [WP, NBLK * BLKW], F16, kind="ExternalInput")
    out_t = nc.dram_tensor("out", [C, 2 * NH, NB], F32, kind="ExternalOutput")

    with tile.TileContext(nc) as tc, ExitStack() as ctx:
        _device_body(tc, ctx, out_t.ap(), ft_t.ap(), msk_t.ap())
    nc.compile()
    return nc


def _prep_ft(feat_n: np.ndarray, h0: int) -> np.ndarray:
    """[C,H,W] -> fT[w'', r, c] fp16 with r over [h0-2, h0+NH+2)."""
    ft = np.zeros((WP, NROWS, C), np.float16)
    r_lo, r_hi = h0 - 2, h0 + NH + 2
    s_lo, s_hi = max(r_lo, 0), min(r_hi, H)
    ft[PAD : PAD + W, s_lo - r_lo : s_hi - r_lo, :] = (
        feat_n[:, s_lo:s_hi, :].transpose(2, 1, 0).astype(np.float16)
    )
    return np.ascontiguousarray(ft.reshape(WP, NROWS * C))


def _prep_msk_full(masks_n: np.ndarray) -> np.ndarray:
    """[25, 2H, 2W] -> full[w', h, i, t20]  [WP, H, KS, RUN]
    t20 = (w - (w'-4))*4 + b*2 + a; value = masks[5i + (4 - t20//4), 2h+a, 2w+b]
    """
    tt = np.arange(RUN)
    wpp = np.arange(WP)
    dw = tt // 4
    b = (tt % 4) // 2
    a = tt % 2
    j = 4 - dw
    wup = 2 * (wpp[:, None] - 4 + dw[None, :]) + b[None, :]
    wup_c = np.clip(wup, 0, 2 * W - 1)                     # [WP, RUN]
    i_ar = np.arange(KS)
    k_full = 5 * i_ar[:, None] + j[None, :]                # [KS, RUN]
    hh = np.arange(H)
    hup = 2 * hh[:, None] + a[None, :]                     # [H, RUN]
    out = masks_n[
        k_full[None, None, :, :],
        hup[None, :, None, :],
        wup_c[:, None, None, :],
    ]  # [WP, H, KS, RUN]
    return out


def _prep_msk_runs(full: np.ndarray, h0: int) -> np.ndarray:
    """band-run stream [WP, NBLK*BLKW] fp16, order (w'', B, i, r, p)."""
    sl = full[:, h0 : h0 + NH]                              # [WP, NH, KS, RUN]
    arr = sl.reshape(WP, NBLK, HB, KS, RUN).transpose(0, 1, 3, 4, 2)
    return np.ascontiguousarray(
        arr.reshape(WP, NBLK * BLKW).astype(np.float16)
    )


def kernel(features: np.ndarray, masks: np.ndarray, _perf: dict | None = None):
    features = np.asarray(features, dtype=np.float32)
    masks = np.asarray(masks, dtype=np.float32)

    if "nc" not in _PROG_CACHE:
        _PROG_CACHE["nc"] = _build_program()
    nc = _PROG_CACHE["nc"]

    in_maps = []
    fulls = [_prep_msk_full(masks[n]) for n in range(N)]
    for core in range(8):
        n, half = divmod(core, 2)
        h0 = NH * half
        in_maps.append(
            {
                "ft": _prep_ft(features[n], h0),
                "mskr": _prep_msk_runs(fulls[n], h0),
            }
        )

    trace = bool(_perf is not None and _perf.get("trace"))
    res = run_bass_kernel_spmd(
        nc, in_maps, core_ids=list(range(8)), trace=trace,
        **({} if not trace else {"trace_cores": [0]}),
    )
    if _perf is not None:
        _perf["exec_time_ns"] = res.exec_time_ns
        _perf["trace"] = res.instructions_and_trace

    out = np.empty((N, C, SCALE * H, SCALE * W), np.float32)
    for core in range(8):
        n, half = divmod(core, 2)
        out[n, :, 64 * half : 64 * half + 64, :] = res.results[core]["out"]
    return out
